# revision 19
# baseline (speedup 1.0000x reference)
"""Distributed Trainium2 kernel for AttHGCNConv:
out = LeakyReLU_0.2( A @ B @ (B.T @ (A.T @ embs)) ),  A=att_adj [N,E], B=inp_adj [E,N].

Never materializes adj = A@B (~1.1 TFLOP); instead chains 4 thin matmuls of
34 GFLOP each — memory-bound. 8-way sharded over the E (hyperedge) axis:
  S1 (local): t1_c = A[:,e_c].T @ embs            lhsT = a_col  [8192,1024]
  S2:  partial = B[e_c,:].T @ t1_c  --AllReduce-> t2 (full, everywhere)
  S3 (local): t3_c = B[e_c,:] @ t2                lhsT = bt_col [8192,1024]
  S4:  partial = A[:,e_c] @ t3_c  --ReduceScatter-> out rows + LeakyReLU

fp16 operands (PSUM accumulates f32), fp16 collective wires. S4 partials are
scaled by 1/16 before the ReduceScatter (final sums reach ~7.6e4 > fp16 max)
and unscaled inside the LeakyReLU epilogue. Collectives are chunked 2-way with
per-chunk bounce tensors so chunk k's collective overlaps chunk k+1's compute.
All weight inputs are host-relaid so every weight DMA is a single 1MB linear
read (sequencer DMA-issue cost, ~0.6us each, was a bottleneck at small sizes).
"""

import sys

for p in ("/opt/trn_rl_repo", "/root/.axon_site"):
    if p not in sys.path:
        sys.path.insert(0, p)

import numpy as np

import concourse.bass as bass  # noqa: F401
import concourse.mybir as mybir
import concourse.tile as tile
from concourse import bacc
from concourse.bass_utils import run_bass_kernel_spmd

N_CORES = 8
N = 8192  # nodes
E = 8192  # hyperedges
D = 256   # embedding dim
S = E // N_CORES   # 1024 per-core E-shard
KT = 128           # partition tile
NK = N // KT       # 64
SK = S // KT       # 8
LEAKY = 0.2

BW_ = 4                      # k/m-tiles fused per weight DMA (1MB each)
NG = NK // BW_               # 16 weight DMAs per matrix
EB = 16                      # embs k-tiles per DMA
T2B = 8                      # t2 k-tiles per DMA

AR_CH = 2                    # AllReduce chunks (t2)
RS_CH = 2                    # ReduceScatter chunks (out)
RS_SCALE = 16.0              # partial4 pre-scale to keep fp16 in range
AR_MG = NG // AR_CH          # weight-DMA groups per AR chunk
RS_GROUPS = [range(0, 12), range(12, 16)]      # uneven: 6144 + 2048 rows
RS_ROWS = [len(r) * BW_ * KT for r in RS_GROUPS]   # [6144, 2048]
RS_SUBS = [r // N_CORES for r in RS_ROWS]          # per-core rows [768, 256]

W16 = mybir.dt.float16       # matmul operand / wire dtype
F32 = mybir.dt.float32
NP16 = np.float16

_CACHED_NC = None


def _build():
    nc = bacc.Bacc("TRN2", target_bir_lowering=False, debug=False,
                   num_devices=N_CORES)

    # all weights pre-fused on host: [NG, 128, BW_*1024], one 1MB DMA per row
    a_g = nc.dram_tensor("a_g", [NG, KT, BW_ * S], W16, kind="ExternalInput")
    b_g = nc.dram_tensor("b_g", [NG, KT, BW_ * S], W16, kind="ExternalInput")
    bt_g = nc.dram_tensor("bt_g", [NG, KT, BW_ * S], W16, kind="ExternalInput")
    at_g = nc.dram_tensor("at_g", [NG, KT, BW_ * S], W16, kind="ExternalInput")
    e_g = nc.dram_tensor("e_g", [NK // EB, KT, EB * D], W16,
                         kind="ExternalInput")
    out = nc.dram_tensor("out", [S, D], F32, kind="ExternalOutput")

    out_v = out.ap().rearrange("(k p) d -> p k d", p=KT)
    rg = [list(range(N_CORES))]

    with tile.TileContext(nc) as tc:
        with (
            tc.tile_pool(name="w", bufs=8) as wpool,
            tc.tile_pool(name="r", bufs=4) as rpool,
            tc.tile_pool(name="keep", bufs=1) as keep,
            tc.tile_pool(name="ev", bufs=4) as evpool,
            tc.tile_pool(name="ps", bufs=8, space="PSUM") as pspool,
            tc.tile_pool(name="dram", bufs=1, space="DRAM") as dram,
        ):
            cc2_ins = [dram.tile([N // AR_CH, D], W16, name=f"cc2_in_{j}",
                                 tag=f"cc2i{j}") for j in range(AR_CH)]
            cc2_outs = [dram.tile([N // AR_CH, D], W16, addr_space="Shared",
                                  name=f"cc2_out_{j}", tag=f"cc2o{j}")
                        for j in range(AR_CH)]
            cc4_ins = [dram.tile([RS_ROWS[j], D], W16, name=f"cc4_in_{j}",
                                 tag=f"cc4i{j}") for j in range(RS_CH)]
            cc4_out = dram.tile([S, D], W16)
            # p-first DRAM views so SBUF APs stay partition-major
            cc2o_vs = [c.rearrange("(g p) d -> p g d", p=KT)
                       for c in cc2_outs]  # [128,32,256] per chunk
            cc2i_vs = [c.rearrange("(g p) d -> p g d", p=KT)
                       for c in cc2_ins]
            cc4i_vs = [c.rearrange("(g p) d -> p g d", p=KT)
                       for c in cc4_ins]
            cc4o_v = cc4_out.rearrange("(k p) d -> p k d", p=KT)

            # ---- S1: t1 = A[:,e_c].T @ embs -> [S, D], kept in SBUF ----
            t1 = keep.tile([KT, SK * D], W16)
            ps1 = [pspool.tile([KT, D], F32, name=f"ps_s1_{m}", tag="ps")
                   for m in range(SK)]
            es = []
            for ge in range(NK // EB):
                er = rpool.tile([KT, EB * D], W16, name="er", tag="r")
                nc.sync.dma_start(er[:], e_g.ap()[ge])
                es.append(er)
            for g in range(NG):
                aw = wpool.tile([KT, BW_ * S], W16, name="aw", tag="w")
                nc.sync.dma_start(aw[:], a_g.ap()[g])
                for kk in range(BW_):
                    k = g * BW_ + kk
                    er = es[k // EB]
                    rh = er[:, (k % EB) * D:(k % EB + 1) * D]
                    for m in range(SK):
                        nc.tensor.matmul(
                            ps1[m][:],
                            aw[:, kk * S + m * KT:kk * S + (m + 1) * KT], rh,
                            start=(k == 0), stop=(k == NK - 1))
            for m in range(SK):
                nc.vector.tensor_copy(t1[:, m * D:(m + 1) * D], ps1[m][:])

            # ---- S2: partial2 = B[e_c,:].T @ t1 -> [N, D] -> AllReduce ----
            for j in range(AR_CH):
                for g in range(j * AR_MG, (j + 1) * AR_MG):
                    bw = wpool.tile([KT, BW_ * S], W16, name="bw", tag="w")
                    nc.sync.dma_start(bw[:], b_g.ap()[g])
                    p2 = evpool.tile([KT, BW_ * D], W16, name="p2", tag="ev")
                    for mm in range(BW_):
                        ps2 = pspool.tile([KT, D], F32, name="ps_s2", tag="ps")
                        for k in range(SK):
                            nc.tensor.matmul(
                                ps2[:],
                                bw[:, mm * S + k * KT:mm * S + (k + 1) * KT],
                                t1[:, k * D:(k + 1) * D],
                                start=(k == 0), stop=(k == SK - 1))
                        nc.vector.tensor_copy(
                            p2[:, mm * D:(mm + 1) * D], ps2[:])
                    lg = g - j * AR_MG
                    nc.scalar.dma_start(
                        cc2i_vs[j][:, lg * BW_:(lg + 1) * BW_, :], p2[:])
                nc.gpsimd.collective_compute(
                    "AllReduce", mybir.AluOpType.add, replica_groups=rg,
                    ins=[cc2_ins[j][:].opt()], outs=[cc2_outs[j][:].opt()])

            # ---- S3: t3 = B[e_c,:] @ t2 -> [S, D], kept in SBUF ----
            t3 = keep.tile([KT, SK * D], W16)
            ps3 = [pspool.tile([KT, D], F32, name=f"ps_s3_{m}", tag="ps")
                   for m in range(SK)]
            t2s = []
            for j in range(AR_CH):
                for gt in range((NK // AR_CH) // T2B):
                    t2r = rpool.tile([KT, T2B * D], W16, name="t2r", tag="r")
                    nc.sync.dma_start(
                        t2r[:], cc2o_vs[j][:, gt * T2B:(gt + 1) * T2B, :])
                    t2s.append(t2r)
            for g in range(NG):
                btw = wpool.tile([KT, BW_ * S], W16, name="btw", tag="w")
                nc.sync.dma_start(btw[:], bt_g.ap()[g])
                for kk in range(BW_):
                    k = g * BW_ + kk
                    t2r = t2s[k // T2B]
                    rh = t2r[:, (k % T2B) * D:(k % T2B + 1) * D]
                    for m in range(SK):
                        nc.tensor.matmul(
                            ps3[m][:],
                            btw[:, kk * S + m * KT:kk * S + (m + 1) * KT], rh,
                            start=(k == 0), stop=(k == NK - 1))
            for m in range(SK):
                nc.vector.tensor_copy(t3[:, m * D:(m + 1) * D], ps3[m][:])

            # ---- S4: partial4 = A[:,e_c] @ t3 * 1/16 -> ReduceScatter ----
            for j in range(RS_CH):
                for g in RS_GROUPS[j]:
                    atw = wpool.tile([KT, BW_ * S], W16, name="atw",
                                     tag="wat", bufs=4)
                    nc.sync.dma_start(atw[:], at_g.ap()[g])
                    p4 = evpool.tile([KT, BW_ * D], W16, name="p4", tag="ev")
                    for mm in range(BW_):
                        ps4 = pspool.tile([KT, D], F32, name="ps_s4", tag="ps")
                        for k in range(SK):
                            nc.tensor.matmul(
                                ps4[:],
                                atw[:, mm * S + k * KT:mm * S + (k + 1) * KT],
                                t3[:, k * D:(k + 1) * D],
                                start=(k == 0), stop=(k == SK - 1))
                        nc.vector.tensor_scalar_mul(
                            p4[:, mm * D:(mm + 1) * D], ps4[:],
                            1.0 / RS_SCALE)
                    lg = g - RS_GROUPS[j][0]
                    nc.scalar.dma_start(
                        cc4i_vs[j][:, lg * BW_:(lg + 1) * BW_, :], p4[:])
                obase = sum(RS_SUBS[:j])
                orows = slice(obase, obase + RS_SUBS[j])
                nc.gpsimd.collective_compute(
                    "ReduceScatter", mybir.AluOpType.add, replica_groups=rg,
                    ins=[cc4_ins[j][:].opt()], outs=[cc4_out[orows, :].opt()])

                # epilogue for this chunk: unscale + LeakyReLU + store
                subk = RS_SUBS[j] // KT
                kb = obase // KT
                o = keep.tile([KT, subk * D], W16, name=f"o_{j}", tag=f"o{j}")
                nc.scalar.dma_start(
                    o[:], cc4o_v[:, kb:kb + subk, :])
                pos = keep.tile([KT, subk * D], F32, name=f"pos_{j}",
                                tag=f"pos{j}")
                neg = keep.tile([KT, subk * D], F32, name=f"neg_{j}",
                                tag=f"neg{j}")
                nc.vector.tensor_scalar_mul(pos[:], o[:], RS_SCALE)
                nc.vector.tensor_scalar_mul(neg[:], o[:], RS_SCALE * LEAKY)
                nc.vector.tensor_max(pos[:], pos[:], neg[:])
                nc.scalar.dma_start(
                    out_v[:, kb:kb + subk, :], pos[:])

    nc.compile()
    return nc


def _fuse(t):
    """[NK,128,F] tile-major -> [NG,128,BW_*F] fused groups (linear DMA)."""
    nk, p, f = t.shape
    return np.ascontiguousarray(
        t.reshape(nk // BW_, BW_, p, f).transpose(0, 2, 1, 3)
    ).reshape(nk // BW_, p, BW_ * f)


def _shard_inputs(inp_adj, att_adj, embs):
    A = np.asarray(att_adj, dtype=np.float32)   # [N, E]
    B = np.asarray(inp_adj, dtype=np.float32)   # [E, N]
    eb = np.asarray(embs, dtype=np.float32).astype(NP16)   # [N, D]
    e_gh = _fuse_e(eb)
    in_maps = []
    for c in range(N_CORES):
        s = slice(c * S, (c + 1) * S)
        a_col = np.ascontiguousarray(A[:, s]).astype(NP16)        # [N, S]
        Bc = B[s, :]                                              # [S, N]
        bt_col = np.ascontiguousarray(Bc.T).astype(NP16)          # [N, S]
        b_m = Bc.reshape(SK, KT, NK, KT).transpose(2, 1, 0, 3) \
            .reshape(NK, KT, S).astype(NP16)
        ATc = A[:, s].T                                           # [S, N]
        at_m = ATc.reshape(SK, KT, NK, KT).transpose(2, 1, 0, 3) \
            .reshape(NK, KT, S).astype(NP16)
        in_maps.append({
            "a_g": _fuse(a_col.reshape(NK, KT, S)),
            "b_g": _fuse(b_m),
            "bt_g": _fuse(bt_col.reshape(NK, KT, S)),
            "at_g": _fuse(at_m),
            "e_g": e_gh,
        })
    return in_maps


def _fuse_e(eb):
    # [N, D] -> [NK/EB, 128, EB*D]
    return np.ascontiguousarray(
        eb.reshape(NK // EB, EB, KT, D).transpose(0, 2, 1, 3)
    ).reshape(NK // EB, KT, EB * D)


def _reset_device():
    """Recover wedged NeuronCores (NRT_EXEC_UNIT_UNRECOVERABLE) via axon."""
    import ctypes

    import jax
    try:
        jax.devices()
        lib = ctypes.CDLL("/opt/axon/libaxon_pjrt.so")
        lib.axon_reset.restype = ctypes.c_int64
        lib.axon_reset()
    except Exception:
        pass


def kernel(inp_adj, att_adj, embs, _trace=False):
    global _CACHED_NC
    if _CACHED_NC is None:
        _CACHED_NC = _build()
    nc = _CACHED_NC
    in_maps = _shard_inputs(inp_adj, att_adj, embs)
    try:
        res = run_bass_kernel_spmd(nc, in_maps,
                                   core_ids=list(range(N_CORES)),
                                   trace=_trace)
    except Exception:
        _reset_device()
        res = run_bass_kernel_spmd(nc, in_maps,
                                   core_ids=list(range(N_CORES)),
                                   trace=_trace)
    # RS chunking scatters rows: chunk j (global row base B_j, per-core size
    # s_j) on core c holds global rows [B_j + c*s_j, +s_j) at local rows
    # [sum(s_<j]) + (0..s_j).
    full = np.empty((N, D), np.float32)
    for c in range(N_CORES):
        oc = res.results[c]["out"]
        for j in range(RS_CH):
            bj = sum(RS_ROWS[:j])
            sj = RS_SUBS[j]
            lb = sum(RS_SUBS[:j])
            full[bj + c * sj: bj + (c + 1) * sj] = oc[lb:lb + sj]
    if _trace:
        kernel.last_exec_time_ns = res.exec_time_ns
    return full


# revision 20
# speedup vs baseline: 1.1095x; 1.1095x over previous
"""Distributed Trainium2 kernel for AttHGCNConv:
out = LeakyReLU_0.2( A @ B @ (B.T @ (A.T @ embs)) ),  A=att_adj [N,E], B=inp_adj [E,N].

Never materializes adj = A@B (~1.1 TFLOP); instead chains 4 thin matmuls of
34 GFLOP each — memory-bound. 8-way sharded over the E (hyperedge) axis:
  S1 (local): t1_c = A[:,e_c].T @ embs            lhsT = a_col  [8192,1024]
  S2:  partial = B[e_c,:].T @ t1_c  --AllReduce-> t2 (full, everywhere)
  S3 (local): t3_c = B[e_c,:] @ t2                lhsT = bt_col [8192,1024]
  S4:  partial = A[:,e_c] @ t3_c  --ReduceScatter-> out rows + LeakyReLU

fp16 operands (PSUM accumulates f32), fp16 collective wires. S4 partials are
scaled by 1/16 before the ReduceScatter (final sums reach ~7.6e4 > fp16 max)
and unscaled inside the LeakyReLU epilogue. Collectives are chunked 2-way with
per-chunk bounce tensors so chunk k's collective overlaps chunk k+1's compute.
All weight inputs are host-relaid so every weight DMA is a single 1MB linear
read (sequencer DMA-issue cost, ~0.6us each, was a bottleneck at small sizes).
"""

import sys

for p in ("/opt/trn_rl_repo", "/root/.axon_site"):
    if p not in sys.path:
        sys.path.insert(0, p)

import numpy as np

import concourse.bass as bass  # noqa: F401
import concourse.mybir as mybir
import concourse.tile as tile
from concourse import bacc
from concourse.bass_utils import run_bass_kernel_spmd

N_CORES = 8
N = 8192  # nodes
E = 8192  # hyperedges
D = 256   # embedding dim
S = E // N_CORES   # 1024 per-core E-shard
KT = 128           # partition tile
NK = N // KT       # 64
SK = S // KT       # 8
LEAKY = 0.2

BW_ = 4                      # k/m-tiles fused per weight DMA (1MB each)
NG = NK // BW_               # 16 weight DMAs per matrix
EB = 16                      # embs k-tiles per DMA
T2B = 8                      # t2 k-tiles per DMA

AR_CH = 2                    # AllReduce chunks (t2)
RS_CH = 2                    # ReduceScatter chunks (out)
RS_SCALE = 16.0              # partial4 pre-scale to keep fp16 in range
AR_MG = NG // AR_CH          # weight-DMA groups per AR chunk
RS_GROUPS = [range(0, 12), range(12, 16)]      # uneven: 6144 + 2048 rows
RS_ROWS = [len(r) * BW_ * KT for r in RS_GROUPS]   # [6144, 2048]
RS_SUBS = [r // N_CORES for r in RS_ROWS]          # per-core rows [768, 256]

W16 = mybir.dt.float16       # matmul operand / wire dtype
F32 = mybir.dt.float32
NP16 = np.float16

_CACHED_NC = None


def _build():
    nc = bacc.Bacc("TRN2", target_bir_lowering=False, debug=False,
                   num_devices=N_CORES)

    # all weights pre-fused on host: [NG, 128, BW_*1024], one 1MB DMA per row
    a_g = nc.dram_tensor("a_g", [NG, KT, BW_ * S], W16, kind="ExternalInput")
    b_g = nc.dram_tensor("b_g", [NG, KT, BW_ * S], W16, kind="ExternalInput")
    bt_g = nc.dram_tensor("bt_g", [NG, KT, BW_ * S], W16, kind="ExternalInput")
    at_g = nc.dram_tensor("at_g", [NG, KT, BW_ * S], W16, kind="ExternalInput")
    e_g = nc.dram_tensor("e_g", [NK // EB, KT, EB * D], W16,
                         kind="ExternalInput")
    out = nc.dram_tensor("out", [S, D], F32, kind="ExternalOutput")

    out_v = out.ap().rearrange("(k p) d -> p k d", p=KT)
    rg = [list(range(N_CORES))]

    with tile.TileContext(nc) as tc:
        with (
            tc.tile_pool(name="w", bufs=8) as wpool,
            tc.tile_pool(name="r", bufs=4) as rpool,
            tc.tile_pool(name="keep", bufs=1) as keep,
            tc.tile_pool(name="ev", bufs=4) as evpool,
            tc.tile_pool(name="ps", bufs=8, space="PSUM") as pspool,
            tc.tile_pool(name="dram", bufs=1, space="DRAM") as dram,
        ):
            cc2_ins = [dram.tile([N // AR_CH, D], W16, name=f"cc2_in_{j}",
                                 tag=f"cc2i{j}") for j in range(AR_CH)]
            cc2_outs = [dram.tile([N // AR_CH, D], W16, addr_space="Shared",
                                  name=f"cc2_out_{j}", tag=f"cc2o{j}")
                        for j in range(AR_CH)]
            cc4_ins = [dram.tile([RS_ROWS[j], D], W16, name=f"cc4_in_{j}",
                                 tag=f"cc4i{j}") for j in range(RS_CH)]
            cc4_out = dram.tile([S, D], W16)
            # p-first DRAM views so SBUF APs stay partition-major
            cc2o_vs = [c.rearrange("(g p) d -> p g d", p=KT)
                       for c in cc2_outs]  # [128,32,256] per chunk
            cc2i_vs = [c.rearrange("(g p) d -> p g d", p=KT)
                       for c in cc2_ins]
            cc4i_vs = [c.rearrange("(g p) d -> p g d", p=KT)
                       for c in cc4_ins]
            cc4o_v = cc4_out.rearrange("(k p) d -> p k d", p=KT)

            # ---- S1: t1 = A[:,e_c].T @ embs -> [S, D], kept in SBUF ----
            t1 = keep.tile([KT, SK * D], W16)
            ps1 = [pspool.tile([KT, D], F32, name=f"ps_s1_{m}", tag="ps")
                   for m in range(SK)]
            es = []
            for ge in range(NK // EB):
                er = rpool.tile([KT, EB * D], W16, name="er", tag="r")
                nc.sync.dma_start(er[:], e_g.ap()[ge])
                es.append(er)
            for g in range(NG):
                aw = wpool.tile([KT, BW_ * S], W16, name="aw", tag="w")
                nc.sync.dma_start(aw[:], a_g.ap()[g])
                for kk in range(BW_):
                    k = g * BW_ + kk
                    er = es[k // EB]
                    rh = er[:, (k % EB) * D:(k % EB + 1) * D]
                    for m in range(SK):
                        nc.tensor.matmul(
                            ps1[m][:],
                            aw[:, kk * S + m * KT:kk * S + (m + 1) * KT], rh,
                            start=(k == 0), stop=(k == NK - 1))
            for m in range(SK):
                nc.vector.tensor_copy(t1[:, m * D:(m + 1) * D], ps1[m][:])

            # ---- S2: partial2 = B[e_c,:].T @ t1 -> [N, D] -> AllReduce ----
            for j in range(AR_CH):
                for g in range(j * AR_MG, (j + 1) * AR_MG):
                    bw = wpool.tile([KT, BW_ * S], W16, name="bw", tag="w")
                    nc.sync.dma_start(bw[:], b_g.ap()[g])
                    p2 = evpool.tile([KT, BW_ * D], W16, name="p2", tag="ev")
                    for mm in range(BW_):
                        ps2 = pspool.tile([KT, D], F32, name="ps_s2", tag="ps")
                        for k in range(SK):
                            nc.tensor.matmul(
                                ps2[:],
                                bw[:, mm * S + k * KT:mm * S + (k + 1) * KT],
                                t1[:, k * D:(k + 1) * D],
                                start=(k == 0), stop=(k == SK - 1))
                        nc.vector.tensor_copy(
                            p2[:, mm * D:(mm + 1) * D], ps2[:])
                    lg = g - j * AR_MG
                    nc.sync.dma_start(
                        cc2i_vs[j][:, lg * BW_:(lg + 1) * BW_, :], p2[:])
                nc.gpsimd.collective_compute(
                    "AllReduce", mybir.AluOpType.add, replica_groups=rg,
                    ins=[cc2_ins[j][:].opt()], outs=[cc2_outs[j][:].opt()])

            # ---- S3: t3 = B[e_c,:] @ t2 -> [S, D], kept in SBUF ----
            t3 = keep.tile([KT, SK * D], W16)
            ps3 = [pspool.tile([KT, D], F32, name=f"ps_s3_{m}", tag="ps")
                   for m in range(SK)]
            t2s = []
            for j in range(AR_CH):
                for gt in range((NK // AR_CH) // T2B):
                    t2r = rpool.tile([KT, T2B * D], W16, name="t2r", tag="r")
                    nc.sync.dma_start(
                        t2r[:], cc2o_vs[j][:, gt * T2B:(gt + 1) * T2B, :])
                    t2s.append(t2r)
            for g in range(NG):
                btw = wpool.tile([KT, BW_ * S], W16, name="btw", tag="w")
                nc.sync.dma_start(btw[:], bt_g.ap()[g])
                for kk in range(BW_):
                    k = g * BW_ + kk
                    t2r = t2s[k // T2B]
                    rh = t2r[:, (k % T2B) * D:(k % T2B + 1) * D]
                    for m in range(SK):
                        nc.tensor.matmul(
                            ps3[m][:],
                            btw[:, kk * S + m * KT:kk * S + (m + 1) * KT], rh,
                            start=(k == 0), stop=(k == NK - 1))
            for m in range(SK):
                nc.vector.tensor_copy(t3[:, m * D:(m + 1) * D], ps3[m][:])

            # ---- S4: partial4 = A[:,e_c] @ t3 * 1/16 -> ReduceScatter ----
            for j in range(RS_CH):
                for g in RS_GROUPS[j]:
                    atw = wpool.tile([KT, BW_ * S], W16, name="atw",
                                     tag="wat", bufs=4)
                    nc.sync.dma_start(atw[:], at_g.ap()[g])
                    p4 = evpool.tile([KT, BW_ * D], W16, name="p4", tag="ev")
                    for mm in range(BW_):
                        ps4 = pspool.tile([KT, D], F32, name="ps_s4", tag="ps")
                        for k in range(SK):
                            nc.tensor.matmul(
                                ps4[:],
                                atw[:, mm * S + k * KT:mm * S + (k + 1) * KT],
                                t3[:, k * D:(k + 1) * D],
                                start=(k == 0), stop=(k == SK - 1))
                        nc.vector.tensor_scalar_mul(
                            p4[:, mm * D:(mm + 1) * D], ps4[:],
                            1.0 / RS_SCALE)
                    lg = g - RS_GROUPS[j][0]
                    nc.sync.dma_start(
                        cc4i_vs[j][:, lg * BW_:(lg + 1) * BW_, :], p4[:])
                obase = sum(RS_SUBS[:j])
                orows = slice(obase, obase + RS_SUBS[j])
                nc.gpsimd.collective_compute(
                    "ReduceScatter", mybir.AluOpType.add, replica_groups=rg,
                    ins=[cc4_ins[j][:].opt()], outs=[cc4_out[orows, :].opt()])

                # epilogue for this chunk: unscale + LeakyReLU + store
                subk = RS_SUBS[j] // KT
                kb = obase // KT
                o = keep.tile([KT, subk * D], W16, name=f"o_{j}", tag=f"o{j}")
                nc.sync.dma_start(
                    o[:], cc4o_v[:, kb:kb + subk, :])
                pos = keep.tile([KT, subk * D], F32, name=f"pos_{j}",
                                tag=f"pos{j}")
                neg = keep.tile([KT, subk * D], F32, name=f"neg_{j}",
                                tag=f"neg{j}")
                nc.vector.tensor_scalar_mul(pos[:], o[:], RS_SCALE)
                nc.vector.tensor_scalar_mul(neg[:], o[:], RS_SCALE * LEAKY)
                nc.vector.tensor_max(pos[:], pos[:], neg[:])
                nc.sync.dma_start(
                    out_v[:, kb:kb + subk, :], pos[:])

    nc.compile()
    return nc


def _fuse(t):
    """[NK,128,F] tile-major -> [NG,128,BW_*F] fused groups (linear DMA)."""
    nk, p, f = t.shape
    return np.ascontiguousarray(
        t.reshape(nk // BW_, BW_, p, f).transpose(0, 2, 1, 3)
    ).reshape(nk // BW_, p, BW_ * f)


def _shard_inputs(inp_adj, att_adj, embs):
    A = np.asarray(att_adj, dtype=np.float32)   # [N, E]
    B = np.asarray(inp_adj, dtype=np.float32)   # [E, N]
    eb = np.asarray(embs, dtype=np.float32).astype(NP16)   # [N, D]
    e_gh = _fuse_e(eb)
    in_maps = []
    for c in range(N_CORES):
        s = slice(c * S, (c + 1) * S)
        a_col = np.ascontiguousarray(A[:, s]).astype(NP16)        # [N, S]
        Bc = B[s, :]                                              # [S, N]
        bt_col = np.ascontiguousarray(Bc.T).astype(NP16)          # [N, S]
        b_m = Bc.reshape(SK, KT, NK, KT).transpose(2, 1, 0, 3) \
            .reshape(NK, KT, S).astype(NP16)
        ATc = A[:, s].T                                           # [S, N]
        at_m = ATc.reshape(SK, KT, NK, KT).transpose(2, 1, 0, 3) \
            .reshape(NK, KT, S).astype(NP16)
        in_maps.append({
            "a_g": _fuse(a_col.reshape(NK, KT, S)),
            "b_g": _fuse(b_m),
            "bt_g": _fuse(bt_col.reshape(NK, KT, S)),
            "at_g": _fuse(at_m),
            "e_g": e_gh,
        })
    return in_maps


def _fuse_e(eb):
    # [N, D] -> [NK/EB, 128, EB*D]
    return np.ascontiguousarray(
        eb.reshape(NK // EB, EB, KT, D).transpose(0, 2, 1, 3)
    ).reshape(NK // EB, KT, EB * D)


def _reset_device():
    """Recover wedged NeuronCores (NRT_EXEC_UNIT_UNRECOVERABLE) via axon."""
    import ctypes

    import jax
    try:
        jax.devices()
        lib = ctypes.CDLL("/opt/axon/libaxon_pjrt.so")
        lib.axon_reset.restype = ctypes.c_int64
        lib.axon_reset()
    except Exception:
        pass


def kernel(inp_adj, att_adj, embs, _trace=False):
    global _CACHED_NC
    if _CACHED_NC is None:
        _CACHED_NC = _build()
    nc = _CACHED_NC
    in_maps = _shard_inputs(inp_adj, att_adj, embs)
    try:
        res = run_bass_kernel_spmd(nc, in_maps,
                                   core_ids=list(range(N_CORES)),
                                   trace=_trace)
    except Exception:
        _reset_device()
        res = run_bass_kernel_spmd(nc, in_maps,
                                   core_ids=list(range(N_CORES)),
                                   trace=_trace)
    # RS chunking scatters rows: chunk j (global row base B_j, per-core size
    # s_j) on core c holds global rows [B_j + c*s_j, +s_j) at local rows
    # [sum(s_<j]) + (0..s_j).
    full = np.empty((N, D), np.float32)
    for c in range(N_CORES):
        oc = res.results[c]["out"]
        for j in range(RS_CH):
            bj = sum(RS_ROWS[:j])
            sj = RS_SUBS[j]
            lb = sum(RS_SUBS[:j])
            full[bj + c * sj: bj + (c + 1) * sj] = oc[lb:lb + sj]
    if _trace:
        kernel.last_exec_time_ns = res.exec_time_ns
    return full
